# revision 1
# baseline (speedup 1.0000x reference)
"""ChildSum TreeLSTM cell on 8 Trainium2 NeuronCores.

Data-parallel over the node axis N: each of the 8 cores processes N/8 nodes.
All feature dims live on SBUF partitions; nodes stream along the free dim.

Host-side prep (free wrt HW time): transpose x/h_msgs/c_msgs to
feature-major [feat, nodes] layouts, cast streams + weights to bf16,
pre-add bias pairs. Device computes, per 1024-node tile:
    wx     = x@Wf.T once, re-injected into each gate PSUM via identity matmul
    gates  f_k = sigmoid(wx + h_k@Uf.T + bf)            (PE + ACT)
    c_tild = sum_k f_k * c_k                            (DVE bf16 tree)
    h_tild = sum_k h_k                                  (DVE bf16 tree)
    iou    = x@Wiou.T + h_tild@Uiou.T + biou            (PE)
    i,o,u  = sigmoid/sigmoid/tanh                       (ACT)
    c = i*u + c_tild ; h = o*tanh(c)                    (DVE + ACT)
"""

import os

os.environ.setdefault("JAX_COMPILATION_CACHE_DIR", "/root/.cache/jax_bass")

import numpy as np
import ml_dtypes

import concourse.bass as bass
import concourse.mybir as mybir
import concourse.tile as tile
from concourse import bacc
from concourse.bass_utils import run_bass_kernel_spmd

BF16 = ml_dtypes.bfloat16
F32 = np.float32

N_CORES = 8
N_FULL = 65536
NSH = N_FULL // N_CORES  # nodes per core
H = 256
X_SIZE = 300
XP = 384  # x feature dim padded to 3*128
K = 4
TN = 1024  # nodes per on-chip tile

SIG = mybir.ActivationFunctionType.Sigmoid
TANH = mybir.ActivationFunctionType.Tanh

LAST_RESULTS = None  # BassKernelResults of the most recent run (for test harness)


def build_bass(nsh=NSH, tn=TN, repeat=1):
    f32 = mybir.dt.float32
    bf = mybir.dt.bfloat16
    nt = nsh // tn
    assert nsh % tn == 0

    nc = bacc.Bacc("TRN2", debug=False)
    nh = tn // 512  # matmul output must stay within one PSUM bank (512 fp32)

    def mm(out_ap, lhsT, rhs, start, stop):
        for s in range(nh):
            ssl = slice(s * 512, (s + 1) * 512)
            nc.tensor.matmul(out_ap[:, ssl], lhsT, rhs[:, ssl], start=start, stop=stop)

    xt = nc.dram_tensor("xt", [3, 128, nsh], bf, kind="ExternalInput")
    ht = nc.dram_tensor("ht", [K, 2, 128, nsh], bf, kind="ExternalInput")
    ct = nc.dram_tensor("ct", [K, 2, 128, nsh], bf, kind="ExternalInput")
    wf = nc.dram_tensor("wf", [3, 128, H], bf, kind="ExternalInput")
    uf = nc.dram_tensor("uf", [2, 128, H], bf, kind="ExternalInput")
    wiou = nc.dram_tensor("wiou", [3, 128, 3 * H], bf, kind="ExternalInput")
    uiou = nc.dram_tensor("uiou", [2, 128, 3 * H], bf, kind="ExternalInput")
    bfb = nc.dram_tensor("bfb", [2, 128], f32, kind="ExternalInput")
    biou = nc.dram_tensor("biou", [6, 128], f32, kind="ExternalInput")
    ident = nc.dram_tensor("ident", [128, 128], bf, kind="ExternalInput")
    # out[0] = h, out[1] = c; chunked [kind, hchunk, 128, nsh]; bf16, host upcasts
    out = nc.dram_tensor("out", [2, 2, 128, nsh], bf, kind="ExternalOutput")

    with tile.TileContext(nc) as tc:
        with (
            tc.tile_pool(name="consts", bufs=1) as consts,
            tc.tile_pool(name="xin", bufs=3) as xin,
            tc.tile_pool(name="hin", bufs=8) as hin,
            tc.tile_pool(name="cin", bufs=8) as cin,
            tc.tile_pool(name="work", bufs=2) as work,
            tc.tile_pool(name="fpool", bufs=4) as fpool,
            tc.tile_pool(name="ppool", bufs=5) as ppool,
            tc.tile_pool(name="accp", bufs=2) as accp,
            tc.tile_pool(name="outp", bufs=3) as outp,
            tc.tile_pool(name="pg", bufs=2, space="PSUM") as pgp,
            tc.tile_pool(name="pio", bufs=2, space="PSUM") as pioup,
        ):
            # gate-critical consts first so the first wx/gate matmuls start early
            wf_s = consts.tile([128, 3, H], bf)
            nc.sync.dma_start(wf_s[:], wf[:].rearrange("c p m -> p c m"))
            uf_s = consts.tile([128, 2, H], bf)
            nc.sync.dma_start(uf_s[:], uf[:].rearrange("c p m -> p c m"))
            id_s = consts.tile([128, 128], bf)
            nc.sync.dma_start(id_s[:], ident[:])
            bfb_s = consts.tile([128, 2], f32)
            nc.sync.dma_start(bfb_s[:], bfb[:].rearrange("c p -> p c"))
            wiou_s = consts.tile([128, 3, 3 * H], bf)
            nc.sync.dma_start(wiou_s[:], wiou[:].rearrange("c p m -> p c m"))
            uiou_s = consts.tile([128, 2, 3 * H], bf)
            nc.sync.dma_start(uiou_s[:], uiou[:].rearrange("c p m -> p c m"))
            biou_s = consts.tile([128, 6], f32)
            nc.sync.dma_start(biou_s[:], biou[:].rearrange("c p -> p c"))

            import contextlib

            rep_ctx = tc.For_i(0, repeat, 1) if repeat > 1 else contextlib.nullcontext()
            with rep_ctx:
              for t in range(nt):
                n0 = t * tn
                nsl = slice(n0, n0 + tn)

                xtile = xin.tile([128, 3, tn], bf, tag="x")
                nc.sync.dma_start(
                    xtile[:], xt[:, :, nsl].rearrange("c p n -> p c n")
                )
                htiles = []
                ctiles = []
                for k in range(K):
                    hk = hin.tile([128, 2, tn], bf, tag="h")
                    nc.sync.dma_start(
                        hk[:], ht[k, :, :, nsl].rearrange("c p n -> p c n")
                    )
                    htiles.append(hk)
                    ck = cin.tile([128, 2, tn], bf, tag="c")
                    nc.sync.dma_start(
                        ck[:], ct[k, :, :, nsl].rearrange("c p n -> p c n")
                    )
                    ctiles.append(ck)

                # h_tild per chunk (bf16 pairwise tree)
                htild = []
                for j in range(2):
                    t01 = work.tile([128, tn], bf, tag="t01")
                    nc.vector.tensor_add(
                        t01[:], htiles[0][:, j, :], htiles[1][:, j, :]
                    )
                    t23 = work.tile([128, tn], bf, tag="t23")
                    nc.vector.tensor_add(
                        t23[:], htiles[2][:, j, :], htiles[3][:, j, :]
                    )
                    hs = work.tile([128, tn], bf, tag="htild")
                    nc.vector.tensor_add(hs[:], t01[:], t23[:])
                    htild.append(hs)

                # wx = x@Wf.T once per chunk -> bf16 SBUF
                wx_sb = []
                for j in range(2):
                    jsl = slice(j * 128, (j + 1) * 128)
                    pwx = pioup.tile([128, tn], f32, tag="pio")
                    for xc in range(3):
                        mm(
                            pwx[:],
                            wf_s[:, xc, jsl],
                            xtile[:, xc, :],
                            start=(xc == 0),
                            stop=(xc == 2),
                        )
                    wxj = work.tile([128, tn], bf, tag="wx")
                    nc.vector.tensor_copy(wxj[:], pwx[:])
                    wx_sb.append(wxj)

                # forget gates + c_tild per chunk
                ctild = []
                for j in range(2):
                    jsl = slice(j * 128, (j + 1) * 128)
                    prods = []
                    for k in range(K):
                        pg = pgp.tile([128, tn], f32, tag="pg")
                        # inject wx via identity matmul, then accumulate uh
                        mm(pg[:], id_s[:], wx_sb[j][:], start=True, stop=False)
                        for hc in range(2):
                            mm(
                                pg[:],
                                uf_s[:, hc, jsl],
                                htiles[k][:, hc, :],
                                start=False,
                                stop=(hc == 1),
                            )
                        fk = fpool.tile([128, tn], bf, tag="f")
                        nc.scalar.activation(
                            fk[:], pg[:], SIG, bias=bfb_s[:, j : j + 1]
                        )
                        pk = ppool.tile([128, tn], bf, tag="p")
                        nc.vector.tensor_mul(pk[:], fk[:], ctiles[k][:, j, :])
                        prods.append(pk)
                    s01 = accp.tile([128, tn], bf, tag="s01")
                    nc.vector.tensor_add(s01[:], prods[0][:], prods[1][:])
                    s23 = accp.tile([128, tn], bf, tag="s23")
                    nc.vector.tensor_add(s23[:], prods[2][:], prods[3][:])
                    cs = accp.tile([128, tn], bf, tag="ctild")
                    nc.vector.tensor_add(cs[:], s01[:], s23[:])
                    ctild.append(cs)

                # iou + outputs per chunk
                for j in range(2):
                    iou_sb = {}
                    for name, oc, func in (
                        ("i", j, SIG),
                        ("o", 2 + j, SIG),
                        ("u", 4 + j, TANH),
                    ):
                        pio = pioup.tile([128, tn], f32, tag="pio")
                        osl = slice(oc * 128, (oc + 1) * 128)
                        for xc in range(3):
                            mm(
                                pio[:],
                                wiou_s[:, xc, osl],
                                xtile[:, xc, :],
                                start=(xc == 0),
                                stop=False,
                            )
                        for hc in range(2):
                            mm(
                                pio[:],
                                uiou_s[:, hc, osl],
                                htild[hc][:],
                                start=False,
                                stop=(hc == 1),
                            )
                        g = fpool.tile([128, tn], bf, tag="g" + name)
                        nc.scalar.activation(
                            g[:], pio[:], func, bias=biou_s[:, oc : oc + 1]
                        )
                        iou_sb[name] = g

                    ciu = outp.tile([128, tn], bf, tag="ciu")
                    nc.vector.tensor_mul(ciu[:], iou_sb["i"][:], iou_sb["u"][:])
                    c_j = outp.tile([128, tn], bf, tag="cout")
                    nc.vector.tensor_add(c_j[:], ciu[:], ctild[j][:])
                    tanh_c = outp.tile([128, tn], bf, tag="tanhc")
                    nc.scalar.activation(tanh_c[:], c_j[:], TANH)
                    h_j = outp.tile([128, tn], bf, tag="hout")
                    nc.vector.tensor_mul(h_j[:], iou_sb["o"][:], tanh_c[:])

                    nc.sync.dma_start(out[0, j, :, nsl], h_j[:])
                    nc.sync.dma_start(out[1, j, :, nsl], c_j[:])

    nc.compile()
    return nc


_NC_CACHE = {}


def _get_nc(nsh, tn):
    key = (nsh, tn)
    if key not in _NC_CACHE:
        _NC_CACHE[key] = build_bass(nsh, tn)
    return _NC_CACHE[key]


def prep_host_inputs(x, h_msgs, c_msgs, W_iou, b_iou, U_iou, b_Uiou, W_f, b_Wf, U_f, b_Uf):
    """Full-input -> per-core input maps (host-side layout only)."""
    n = x.shape[0]
    nsh = n // N_CORES

    xp = np.zeros((XP, n), F32)
    xp[:X_SIZE] = x.T
    xt_full = np.ascontiguousarray(xp).astype(BF16).reshape(3, 128, n)

    ht_full = np.ascontiguousarray(h_msgs.astype(BF16).transpose(1, 2, 0)).reshape(
        K, 2, 128, n
    )
    ct_full = np.ascontiguousarray(c_msgs.astype(BF16).transpose(1, 2, 0)).reshape(
        K, 2, 128, n
    )

    wfp = np.zeros((XP, H), F32)
    wfp[:X_SIZE] = W_f.T
    wf_host = wfp.astype(BF16).reshape(3, 128, H)
    uf_host = np.ascontiguousarray(U_f.T).astype(BF16).reshape(2, 128, H)
    wioup = np.zeros((XP, 3 * H), F32)
    wioup[:X_SIZE] = W_iou.T
    wiou_host = wioup.astype(BF16).reshape(3, 128, 3 * H)
    uiou_host = np.ascontiguousarray(U_iou.T).astype(BF16).reshape(2, 128, 3 * H)

    bfb_host = (b_Wf + b_Uf).astype(F32).reshape(2, 128)
    biou_host = (b_iou + b_Uiou).astype(F32).reshape(6, 128)
    ident_host = np.eye(128, dtype=F32).astype(BF16)

    in_maps = []
    for c in range(N_CORES):
        sl = slice(c * nsh, (c + 1) * nsh)
        in_maps.append(
            {
                "xt": np.ascontiguousarray(xt_full[:, :, sl]),
                "ht": np.ascontiguousarray(ht_full[:, :, :, sl]),
                "ct": np.ascontiguousarray(ct_full[:, :, :, sl]),
                "wf": wf_host,
                "uf": uf_host,
                "wiou": wiou_host,
                "uiou": uiou_host,
                "bfb": bfb_host,
                "biou": biou_host,
                "ident": ident_host,
            }
        )
    return in_maps


def kernel(**inputs):
    global LAST_RESULTS
    inputs = {k: np.asarray(v) for k, v in inputs.items()}
    n = inputs["x"].shape[0]
    assert n == N_FULL, f"hardcoded for N={N_FULL}, got {n}"
    nsh = n // N_CORES

    nc = _get_nc(nsh, TN)
    in_maps = prep_host_inputs(**inputs)

    res = None
    for attempt in range(3):
        try:
            res = run_bass_kernel_spmd(nc, in_maps, core_ids=list(range(N_CORES)))
            break
        except Exception:
            if attempt == 2:
                raise
            import time as _time

            _time.sleep(5.0)
    LAST_RESULTS = res

    # results[c]["out"]: [2, 2, 128, nsh] -> full [2, N, 256]
    per_core = [r["out"].astype(F32).reshape(2, 256, nsh) for r in res.results]
    full = np.concatenate(per_core, axis=-1)  # [2, 256, N]
    return np.ascontiguousarray(full.transpose(0, 2, 1)).astype(F32)



# revision 10
# speedup vs baseline: 1.0010x; 1.0010x over previous
"""ChildSum TreeLSTM cell on 8 Trainium2 NeuronCores.

Data-parallel over the node axis N: each of the 8 cores processes N/8 nodes.
Feature dims live on SBUF partitions (2 chunks of 128 for H=256); nodes
stream along the free dim.

v2: fp8(e4m3) matmuls with DoubleRow (contraction 256 in one pass),
biases folded into x's padding row (x[300]=1, W[300]=bias*S), weights
pre-scaled by S=16 and un-scaled for free via the ACT `scale` field.
Per 1024-node tile, split into two 512-column PSUM steps:
    wx     = x@Wf.T            (PE: 1 DR + 1 plain mm per chunk)
    f_in   = wx + h_k@Uf.T     (PE DR + DVE add into batched SBUF tile)
    f      = sigmoid(f_in/S)   (ACT: one batched call per tile)
    h_tild = sum_k h_k         (GPSIMD fp8 tree)
    c_tild = sum_k f_k * c_k   (DVE bf16)
    iou    = x@Wiou.T + h_tild@Uiou.T   (PE DR)
    i,o,u  = sig/sig/tanh(iou/S)        (ACT, chunk-batched)
    c = i*u + c_tild ; h = o*tanh(c)    (GPSIMD/DVE + ACT)
"""

import os

os.environ.setdefault("JAX_COMPILATION_CACHE_DIR", "/root/.cache/jax_bass")

import numpy as np
import ml_dtypes

import concourse.bass as bass
import concourse.mybir as mybir
import concourse.tile as tile
from concourse import bacc
from concourse.bass_utils import run_bass_kernel_spmd

BF16 = ml_dtypes.bfloat16
FP8 = ml_dtypes.float8_e4m3
F32 = np.float32

N_CORES = 8
N_FULL = 65536
NSH = N_FULL // N_CORES  # nodes per core
H = 256
X_SIZE = 300
XP = 384  # x feature dim padded to 3*128 (row 300 = 1.0 carries the bias)
K = 4
TN = 1024  # nodes per on-chip tile
WS = 16.0  # weight pre-scale (fp8 range centering); undone by ACT scale

SIG = mybir.ActivationFunctionType.Sigmoid
TANH = mybir.ActivationFunctionType.Tanh
DR = mybir.MatmulPerfMode.DoubleRow

LAST_RESULTS = None  # BassKernelResults of the most recent run (for test harness)


def build_bass(nsh=NSH, tn=TN):
    f32 = mybir.dt.float32
    bf = mybir.dt.bfloat16
    f8 = mybir.dt.float8e4
    nt = nsh // tn
    assert nsh % tn == 0
    ns = tn // 512  # 512-column PSUM steps per tile

    nc = bacc.Bacc("TRN2", debug=False)

    xt = nc.dram_tensor("xt", [3, 128, nsh], f8, kind="ExternalInput")
    ht = nc.dram_tensor("ht", [K, 2, 128, nsh], f8, kind="ExternalInput")
    ct = nc.dram_tensor("ct", [K, 2, 128, nsh], bf, kind="ExternalInput")
    wf = nc.dram_tensor("wf", [3, 128, H], f8, kind="ExternalInput")
    uf = nc.dram_tensor("uf", [2, 128, H], f8, kind="ExternalInput")
    wiou = nc.dram_tensor("wiou", [3, 128, 3 * H], f8, kind="ExternalInput")
    uiou = nc.dram_tensor("uiou", [2, 128, 3 * H], f8, kind="ExternalInput")
    # out[0] = h, out[1] = c; chunked [kind, hchunk, 128, nsh]; bf16, host upcasts
    out = nc.dram_tensor("out", [2, 2, 128, nsh], bf, kind="ExternalOutput")

    inv = 1.0 / WS

    with tile.TileContext(nc) as tc:
        with (
            tc.tile_pool(name="consts", bufs=1) as consts,
            tc.tile_pool(name="xin", bufs=3) as xin,
            tc.tile_pool(name="hin", bufs=8) as hin,
            tc.tile_pool(name="cin", bufs=6) as cin,
            tc.tile_pool(name="wxp", bufs=3) as wxp,
            tc.tile_pool(name="finp", bufs=2) as finp,
            tc.tile_pool(name="htp", bufs=2) as htp,
            tc.tile_pool(name="ctp", bufs=3) as ctp,
            tc.tile_pool(name="gio", bufs=4) as gio,
            tc.tile_pool(name="outp", bufs=4) as outp,
            tc.tile_pool(name="ps2", bufs=2, space="PSUM") as ps2,
            tc.tile_pool(name="psio", bufs=2, space="PSUM") as psio,
        ):
            # gate-critical consts first so the first wx/gate matmuls start early
            wf_s = consts.tile([128, 3, H], f8)
            nc.sync.dma_start(wf_s[:], wf[:].rearrange("c p m -> p c m"))
            uf_s = consts.tile([128, 2, H], f8)
            nc.sync.dma_start(uf_s[:], uf[:].rearrange("c p m -> p c m"))
            wiou_s = consts.tile([128, 3, 3 * H], f8)
            nc.sync.dma_start(wiou_s[:], wiou[:].rearrange("c p m -> p c m"))
            uiou_s = consts.tile([128, 2, 3 * H], f8)
            nc.sync.dma_start(uiou_s[:], uiou[:].rearrange("c p m -> p c m"))

            for t in range(nt):
                n0 = t * tn
                nsl = slice(n0, n0 + tn)

                xtile = xin.tile([128, 3, tn], f8, tag="x")
                nc.sync.dma_start(
                    xtile[:], xt[:, :, nsl].rearrange("c p n -> p c n")
                )
                htiles = []
                ctiles = []
                for k in range(K):
                    hk = hin.tile([128, 2, tn], f8, tag="h")
                    nc.sync.dma_start(
                        hk[:], ht[k, :, :, nsl].rearrange("c p n -> p c n")
                    )
                    htiles.append(hk)
                    ck = cin.tile([128, 2, tn], bf, tag="c")
                    nc.sync.dma_start(
                        ck[:], ct[k, :, :, nsl].rearrange("c p n -> p c n")
                    )
                    ctiles.append(ck)

                # h_tild (fp8 in -> fp8 out, for the DR U_iou matmul); GPSIMD
                t01 = htp.tile([128, 2, tn], f8, tag="t01")
                nc.gpsimd.tensor_add(t01[:], htiles[0][:], htiles[1][:])
                t23 = htp.tile([128, 2, tn], f8, tag="t23")
                nc.gpsimd.tensor_add(t23[:], htiles[2][:], htiles[3][:])
                htild = htp.tile([128, 2, tn], f8, tag="htild")
                nc.gpsimd.tensor_add(htild[:], t01[:], t23[:])

                # batched f-gate pre-activations for the whole tile
                # (sigmoid applied in place: one elementwise ACT instruction)
                fin = finp.tile([128, ns * 8, 512], bf, tag="fin")
                f_sb = fin

                wx_sbs = []
                for s in range(ns):
                    ssl = slice(s * 512, (s + 1) * 512)

                    # wx = x@Wf.T (scaled); 1 DR + 1 plain matmul per chunk
                    pwx = ps2.tile([128, 2, 512], f32, tag="pg")
                    for j in range(2):
                        jsl = slice(j * 128, (j + 1) * 128)
                        nc.tensor.matmul(
                            pwx[:, j, :],
                            wf_s[:, 0:2, jsl],
                            xtile[:, 0:2, ssl],
                            start=True,
                            stop=False,
                            perf_mode=DR,
                        )
                        nc.tensor.matmul(
                            pwx[:, j, :],
                            wf_s[:, 2, jsl],
                            xtile[:, 2, ssl],
                            start=False,
                            stop=True,
                        )
                    wx_sb = wxp.tile([128, 2, 512], bf, tag="wx")
                    nc.vector.tensor_copy(wx_sb[:], pwx[:])
                    wx_sbs.append(wx_sb)

                    # forget-gate pre-activations: uh_k + wx -> fin slots
                    for k in range(K):
                        pg = ps2.tile([128, 2, 512], f32, tag="pg")
                        for j in range(2):
                            jsl = slice(j * 128, (j + 1) * 128)
                            nc.tensor.matmul(
                                pg[:, j, :],
                                uf_s[:, :, jsl],
                                htiles[k][:, :, ssl],
                                start=True,
                                stop=True,
                                perf_mode=DR,
                            )
                        nc.vector.tensor_add(
                            fin[:, s * 8 + 2 * k : s * 8 + 2 * k + 2, :],
                            pg[:],
                            wx_sb[:],
                        )

                # one sigmoid for all 4 gates x 2 chunks x ns steps
                nc.scalar.activation(f_sb[:], fin[:], SIG, scale=inv)

                for s in range(ns):
                    ssl = slice(s * 512, (s + 1) * 512)

                    # c_tild = sum_k f_k * c_k   (DVE bf16)
                    prods = []
                    for k in range(K):
                        pk = ctp.tile([128, 2, 512], bf, tag="p")
                        nc.vector.tensor_mul(
                            pk[:],
                            f_sb[:, s * 8 + 2 * k : s * 8 + 2 * k + 2, :],
                            ctiles[k][:, :, ssl],
                        )
                        prods.append(pk)
                    s01 = ctp.tile([128, 2, 512], bf, tag="s01")
                    nc.vector.tensor_add(s01[:], prods[0][:], prods[1][:])
                    s23 = ctp.tile([128, 2, 512], bf, tag="s23")
                    nc.vector.tensor_add(s23[:], prods[2][:], prods[3][:])
                    ctild = ctp.tile([128, 2, 512], bf, tag="ctild")
                    nc.vector.tensor_add(ctild[:], s01[:], s23[:])

                    # iou: per chunk j, gates (i,o) batched in one 2-bank PSUM
                    # + u in its own; x-side 1 DR + 1 plain, h-side 1 DR each
                    def iou_mms(pdst, oc):
                        osl = slice(oc * 128, (oc + 1) * 128)
                        nc.tensor.matmul(
                            pdst,
                            wiou_s[:, 0:2, osl],
                            xtile[:, 0:2, ssl],
                            start=True,
                            stop=False,
                            perf_mode=DR,
                        )
                        nc.tensor.matmul(
                            pdst,
                            wiou_s[:, 2, osl],
                            xtile[:, 2, ssl],
                            start=False,
                            stop=False,
                        )
                        nc.tensor.matmul(
                            pdst,
                            uiou_s[:, :, osl],
                            htild[:, :, ssl],
                            start=False,
                            stop=True,
                            perf_mode=DR,
                        )

                    gi = {}
                    for j in range(2):
                        pio = psio.tile([128, 2, 512], f32, tag="pio")
                        iou_mms(pio[:, 0, :], j)          # i (chunk j)
                        iou_mms(pio[:, 1, :], 2 + j)      # o (chunk j)
                        g_io = gio.tile([128, 2, 512], bf, tag="gio")
                        nc.scalar.activation(g_io[:], pio[:], SIG, scale=inv)
                        gi[("i", j)] = g_io[:, 0, :]
                        gi[("o", j)] = g_io[:, 1, :]
                    pu = psio.tile([128, 2, 512], f32, tag="pio")
                    iou_mms(pu[:, 0, :], 4)               # u chunk 0
                    iou_mms(pu[:, 1, :], 5)               # u chunk 1
                    g_u = gio.tile([128, 2, 512], bf, tag="gu")
                    nc.scalar.activation(g_u[:], pu[:], TANH, scale=inv)

                    # outputs: c = i*u + ctild; h = o*tanh(c)   (GPSIMD + ACT)
                    ciu = outp.tile([128, 2, 512], bf, tag="ciu")
                    for j in range(2):
                        nc.gpsimd.tensor_mul(
                            ciu[:, j, :], gi[("i", j)], g_u[:, j, :]
                        )
                    c_t = outp.tile([128, 2, 512], bf, tag="cout")
                    nc.gpsimd.tensor_add(c_t[:], ciu[:], ctild[:])
                    tanh_c = outp.tile([128, 2, 512], bf, tag="tanhc")
                    nc.scalar.activation(tanh_c[:], c_t[:], TANH)
                    h_t = outp.tile([128, 2, 512], bf, tag="hout")
                    for j in range(2):
                        nc.gpsimd.tensor_mul(
                            h_t[:, j, :], gi[("o", j)], tanh_c[:, j, :]
                        )

                    nsl512 = slice(n0 + s * 512, n0 + (s + 1) * 512)
                    nc.sync.dma_start(
                        out[0, :, :, nsl512].rearrange("c p n -> p c n"), h_t[:]
                    )
                    nc.sync.dma_start(
                        out[1, :, :, nsl512].rearrange("c p n -> p c n"), c_t[:]
                    )

    nc.compile()
    return nc


_NC_CACHE = {}


def _get_nc(nsh, tn):
    key = (nsh, tn)
    if key not in _NC_CACHE:
        _NC_CACHE[key] = build_bass(nsh, tn)
    return _NC_CACHE[key]


def prep_host_inputs(x, h_msgs, c_msgs, W_iou, b_iou, U_iou, b_Uiou, W_f, b_Wf, U_f, b_Uf):
    """Full-input -> per-core input maps (host-side layout only)."""
    n = x.shape[0]
    nsh = n // N_CORES

    xp = np.zeros((XP, n), F32)
    xp[:X_SIZE] = x.T
    xp[X_SIZE] = 1.0  # constant row carrying the bias through the matmul
    xt_full = np.ascontiguousarray(xp).astype(FP8).reshape(3, 128, n)

    ht_full = np.ascontiguousarray(h_msgs.astype(FP8).transpose(1, 2, 0)).reshape(
        K, 2, 128, n
    )
    ct_full = np.ascontiguousarray(c_msgs.astype(BF16).transpose(1, 2, 0)).reshape(
        K, 2, 128, n
    )

    wfp = np.zeros((XP, H), F32)
    wfp[:X_SIZE] = W_f.T * WS
    wfp[X_SIZE] = (b_Wf + b_Uf) * WS
    wf_host = wfp.astype(FP8).reshape(3, 128, H)
    uf_host = np.ascontiguousarray(U_f.T * WS).astype(FP8).reshape(2, 128, H)
    wioup = np.zeros((XP, 3 * H), F32)
    wioup[:X_SIZE] = W_iou.T * WS
    wioup[X_SIZE] = (b_iou + b_Uiou) * WS
    wiou_host = wioup.astype(FP8).reshape(3, 128, 3 * H)
    uiou_host = np.ascontiguousarray(U_iou.T * WS).astype(FP8).reshape(2, 128, 3 * H)

    in_maps = []
    for c in range(N_CORES):
        sl = slice(c * nsh, (c + 1) * nsh)
        in_maps.append(
            {
                "xt": np.ascontiguousarray(xt_full[:, :, sl]),
                "ht": np.ascontiguousarray(ht_full[:, :, :, sl]),
                "ct": np.ascontiguousarray(ct_full[:, :, :, sl]),
                "wf": wf_host,
                "uf": uf_host,
                "wiou": wiou_host,
                "uiou": uiou_host,
            }
        )
    return in_maps


def kernel(**inputs):
    global LAST_RESULTS
    inputs = {k: np.asarray(v) for k, v in inputs.items()}
    n = inputs["x"].shape[0]
    assert n == N_FULL, f"hardcoded for N={N_FULL}, got {n}"
    nsh = n // N_CORES

    nc = _get_nc(nsh, TN)
    in_maps = prep_host_inputs(**inputs)

    res = None
    for attempt in range(3):
        try:
            res = run_bass_kernel_spmd(nc, in_maps, core_ids=list(range(N_CORES)))
            break
        except Exception:
            if attempt == 2:
                raise
            import time as _time

            _time.sleep(5.0)
    LAST_RESULTS = res

    # results[c]["out"]: [2, 2, 128, nsh] -> full [2, N, 256]
    per_core = [r["out"].astype(F32).reshape(2, 256, nsh) for r in res.results]
    full = np.concatenate(per_core, axis=-1)  # [2, 256, N]
    return np.ascontiguousarray(full.transpose(0, 2, 1)).astype(F32)
